# revision 1
# baseline (speedup 1.0000x reference)
"""LoRA first-layer MLP kernel for 8 Trainium2 NeuronCores.

Computation:
    W_eff = W0 + 2.0 * (B @ A)            # [4096, 1024]
    h     = relu(x @ W_eff^T + b0)        # [16384, 4096]
    out   = (h @ W2^T + b2).squeeze(-1)   # [16384]

Sharding: data-parallel over batch; each of the 8 cores handles 2048 rows of
x and replicates the weights. No collectives needed.

Per-core device kernel (fp32 data, fp32r matmul mode):
  - W0^T streamed to SBUF in [mc2(8), dc(8), 128, 512] blocks; the LoRA
    rank-16 correction 2*(B@A)^T is pre-added into each block on-device
    (PE matmul with zero-padded K=128 operands -> PSUM, DVE add),
    emitted just-in-time ahead of the tiles that read the block.
  - Layer 1: h^T[m, b] tiles [128, 512] accumulated on PE over 8 d-chunks
    (lhsT = W_eff^T slice [128d, 128m], rhs = x^T slice [128d, 512b]).
  - relu+bias on ScalarE (bias b0 is per-partition in this layout).
  - Layer 2 (sum_m W2[m]*h[m,b]) off the PE: even m-tiles accumulate on
    VectorE (scalar_tensor_tensor), odd tiles on GpSimdE (mul+add with a
    free-broadcast W2); final partition-reduce via two ones-vector
    matmuls per batch chunk, deferred into the next chunk's stream.
"""

import sys

sys.path.insert(0, "/opt/trn_rl_repo")

import numpy as np

import concourse.bacc as bacc
import concourse.bass as bass
import concourse.mybir as mybir
import concourse.tile as tile
from concourse.bass_utils import run_bass_kernel_spmd

F32 = mybir.dt.float32
F32R = mybir.dt.float32r

N_CORES = 8
B_FULL, D, M, R = 16384, 1024, 4096, 16
SCALING = 2.0
BS = B_FULL // N_CORES  # 2048 rows per core
NB = BS // 512  # 4 batch chunks per core
ND = D // 128  # 8 d-chunks
NM = M // 128  # 32 m-chunks
NM2 = M // 512  # 8 m-blocks of 512

_CACHE = {}


def _build_nc():
    nc = bacc.Bacc(
        "TRN2",
        target_bir_lowering=False,
        debug=False,
        num_devices=N_CORES,
    )
    xt = nc.dram_tensor("xt", [NB, 128, ND * 512], F32R, kind="ExternalInput").ap()
    w0t = nc.dram_tensor("w0t", [NM2, ND, 128, 512], F32R, kind="ExternalInput").ap()
    a2p = nc.dram_tensor("a2p", [128, D], F32R, kind="ExternalInput").ap()
    btp = nc.dram_tensor("btp", [128, M], F32R, kind="ExternalInput").ap()
    b0c = nc.dram_tensor("b0c", [128, NM], F32, kind="ExternalInput").ap()
    w2c = nc.dram_tensor("w2c", [128, NM], F32, kind="ExternalInput").ap()
    b2s = nc.dram_tensor("b2s", [1, 1], F32, kind="ExternalInput").ap()
    onesd = nc.dram_tensor("ones", [128, 1], F32R, kind="ExternalInput").ap()
    out = nc.dram_tensor("out", [1, BS], F32, kind="ExternalOutput").ap()

    RELU = mybir.ActivationFunctionType.Relu
    MULT = mybir.AluOpType.mult
    ADD = mybir.AluOpType.add

    with tile.TileContext(nc) as tc:
        with (
            tc.tile_pool(name="wp", bufs=1) as wp,
            tc.tile_pool(name="xp", bufs=2) as xp,
            tc.tile_pool(name="hb", bufs=4) as hb,
            tc.tile_pool(name="hw2", bufs=2) as hw2,
            tc.tile_pool(name="ab", bufs=2) as ab,
            tc.tile_pool(name="cp", bufs=1) as cp,
            tc.tile_pool(name="psh", bufs=3, space="PSUM") as psh,
            tc.tile_pool(name="pso", bufs=2, space="PSUM") as pso,
            tc.tile_pool(name="psl", bufs=3, space="PSUM") as psl,
        ):
            A2 = cp.tile([128, D], F32R, tag="a2")
            nc.sync.dma_start(out=A2[:], in_=a2p)

            # First x chunk interleaved with the first W m-block on sync.
            xb0 = xp.tile([128, ND * 512], F32R, tag="xb", name="xb0")

            def xb0_dma(dc):
                nc.sync.dma_start(
                    out=xb0[:, dc * 512 : (dc + 1) * 512],
                    in_=xt[0][:, dc * 512 : (dc + 1) * 512],
                )

            BT = cp.tile([128, M], F32R, tag="bt")
            for i in range(8):
                eng = nc.gpsimd if i % 2 == 0 else nc.scalar
                eng.dma_start(
                    out=BT[:, i * 512 : (i + 1) * 512],
                    in_=btp[:, i * 512 : (i + 1) * 512],
                )

            # Resident W_eff^T, laid out [mc2, dc, 512] along the free dim.
            W = wp.tile([128, NM2 * ND * 512], F32R, tag="w")

            def w_dma(mc2s):
                for mc2 in mc2s:
                    for dc in range(ND):
                        blk = (mc2 * ND + dc) * 512
                        nc.sync.dma_start(
                            out=W[:, blk : blk + 512], in_=w0t[mc2, dc]
                        )

            # xb0 slice dc, then W block (0, dc): compute needs both pairwise.
            for dc in range(ND):
                xb0_dma(dc)
                blk = dc * 512
                nc.sync.dma_start(out=W[:, blk : blk + 512], in_=w0t[0, dc])
            B0 = cp.tile([128, NM], F32, tag="b0")
            nc.sync.dma_start(out=B0[:], in_=b0c)
            W2 = cp.tile([128, NM], F32, tag="w2")
            nc.sync.dma_start(out=W2[:], in_=w2c)
            B2 = cp.tile([1, 1], F32, tag="b2")
            nc.sync.dma_start(out=B2[:], in_=b2s)
            ONES = cp.tile([128, 1], F32R, tag="ones")
            nc.sync.dma_start(out=ONES[:], in_=onesd)
            w_dma(range(1, NM2))

            def lora_block(mc2, dc):
                """W[:, blk:blk+512] += 2*(B@A)^T block, just-in-time."""
                blk = (mc2 * ND + dc) * 512
                lp = psl.tile([128, 512], F32, tag="lp")
                nc.tensor.matmul(
                    lp[:],
                    A2[:, dc * 128 : (dc + 1) * 128],
                    BT[:, mc2 * 512 : (mc2 + 1) * 512],
                    start=True,
                    stop=True,
                )
                nc.vector.tensor_add(
                    W[:, blk : blk + 512], W[:, blk : blk + 512], lp[:]
                )

            pending_reduce = []

            def emit_reduce(bc, acc_e, acc_o):
                op = pso.tile([1, 512], F32, tag="op")
                nc.tensor.matmul(op[:], ONES[:], acc_e[:], start=True, stop=False)
                nc.tensor.matmul(op[:], ONES[:], acc_o[:], start=False, stop=True)
                os_t = ab.tile([1, 512], F32, tag="os")
                nc.vector.tensor_scalar_add(os_t[:], op[:], B2[:, 0:1])
                nc.sync.dma_start(
                    out=out[:, bc * 512 : (bc + 1) * 512], in_=os_t[:]
                )

            for bc in range(NB):
                if bc == 0:
                    xb = xb0
                    # LoRA group 0 up front (paced by W DMA anyway)
                    for dc in range(ND):
                        lora_block(0, dc)
                else:
                    xb = xp.tile([128, ND * 512], F32R, tag="xb")
                    for dc in range(ND):
                        nc.sync.dma_start(
                            out=xb[:, dc * 512 : (dc + 1) * 512],
                            in_=xt[bc][:, dc * 512 : (dc + 1) * 512],
                        )
                acc_e = ab.tile([128, 512], F32R, tag="acce")
                acc_o = ab.tile([128, 512], F32R, tag="acco")
                for mc in range(NM):
                    mc2, j0 = mc // 4, (mc % 4) * 128
                    if bc == 0:
                        # prefetch next group's LoRA blocks, 2 per tile
                        g_next = mc // 4 + 1
                        if g_next < NM2:
                            for dc in (2 * (mc % 4), 2 * (mc % 4) + 1):
                                lora_block(g_next, dc)
                    if mc == 2 and pending_reduce:
                        emit_reduce(*pending_reduce.pop())
                    hp = psh.tile([128, 512], F32, tag="hp")
                    for dc in range(ND):
                        blk = (mc2 * ND + dc) * 512 + j0
                        nc.tensor.matmul(
                            hp[:],
                            W[:, blk : blk + 128],
                            xb[:, dc * 512 : (dc + 1) * 512],
                            start=(dc == 0),
                            stop=(dc == ND - 1),
                        )
                    h = hb.tile([128, 512], F32, tag="h")
                    nc.scalar.activation(h[:], hp[:], RELU, bias=B0[:, mc : mc + 1])
                    # acc += h * W2[m]: even tiles on VectorE (fused stt),
                    # odd tiles on GpSimd (mult into scratch, then add).
                    if mc % 2 == 0:
                        if mc == 0:
                            nc.vector.tensor_scalar_mul(
                                acc_e[:], h[:], W2[:, mc : mc + 1]
                            )
                        else:
                            nc.vector.scalar_tensor_tensor(
                                acc_e[:], h[:], W2[:, mc : mc + 1], acc_e[:],
                                MULT, ADD,
                            )
                    elif bc == NB - 1 and mc >= NM - 3:
                        nc.vector.scalar_tensor_tensor(
                            acc_o[:], h[:], W2[:, mc : mc + 1], acc_o[:],
                            MULT, ADD,
                        )
                    else:
                        w2b = W2[:, mc : mc + 1].broadcast_to([128, 512])
                        if mc == 1:
                            nc.gpsimd.tensor_mul(acc_o[:], h[:], w2b)
                        else:
                            hw = hw2.tile([128, 512], F32, tag="hw")
                            nc.gpsimd.tensor_mul(hw[:], h[:], w2b)
                            nc.gpsimd.tensor_add(acc_o[:], acc_o[:], hw[:])
                pending_reduce.append((bc, acc_e, acc_o))
            while pending_reduce:
                emit_reduce(*pending_reduce.pop(0))

    nc.compile()
    return nc


def _prep_in_maps(x, W0, b0, A, B, W2, b2):
    w0t_full = np.ascontiguousarray(W0.T).reshape(ND, 128, M)
    # -> [mc2, dc, 128, 512]
    w0t = np.ascontiguousarray(
        w0t_full.reshape(ND, 128, NM2, 512).transpose(2, 0, 1, 3)
    )
    a2p = np.zeros((128, D), dtype=np.float32)
    a2p[:R] = SCALING * A
    btp = np.zeros((128, M), dtype=np.float32)
    btp[:R] = B.T
    b0c = np.ascontiguousarray(b0.reshape(NM, 128).T)
    w2c = np.ascontiguousarray(W2[0].reshape(NM, 128).T)
    b2s = b2.reshape(1, 1).astype(np.float32)
    ones = np.ones((128, 1), dtype=np.float32)

    in_maps = []
    for c in range(N_CORES):
        xs = x[c * BS : (c + 1) * BS]  # [2048, 1024]
        # xt[bc, p, dc*512 + b] = xs[bc*512 + b, dc*128 + p]
        xt = np.ascontiguousarray(
            xs.reshape(NB, 512, ND, 128).transpose(0, 3, 2, 1).reshape(NB, 128, ND * 512)
        )
        in_maps.append(
            {
                "xt": xt,
                "w0t": w0t,
                "a2p": a2p,
                "btp": btp,
                "b0c": b0c,
                "w2c": w2c,
                "b2s": b2s,
                "ones": ones,
            }
        )
    return in_maps


def kernel(x, W0, b0, A, B, W2, b2, _trace=False, _trace_kwargs=None):
    x = np.asarray(x, dtype=np.float32)
    W0 = np.asarray(W0, dtype=np.float32)
    b0 = np.asarray(b0, dtype=np.float32)
    A = np.asarray(A, dtype=np.float32)
    B = np.asarray(B, dtype=np.float32)
    W2 = np.asarray(W2, dtype=np.float32)
    b2 = np.asarray(b2, dtype=np.float32)

    if "nc" not in _CACHE:
        _CACHE["nc"] = _build_nc()
    nc = _CACHE["nc"]

    in_maps = _prep_in_maps(x, W0, b0, A, B, W2, b2)
    res = run_bass_kernel_spmd(
        nc,
        in_maps,
        list(range(N_CORES)),
        trace=_trace,
        **(_trace_kwargs or {}),
    )
    out = np.concatenate([r["out"].reshape(BS) for r in res.results])
    if _trace:
        _CACHE["last_results"] = res
    return out.astype(np.float32)



# revision 4
# speedup vs baseline: 1.0524x; 1.0524x over previous
"""LoRA first-layer MLP kernel for 8 Trainium2 NeuronCores.

Computation:
    W_eff = W0 + 2.0 * (B @ A)            # [4096, 1024]
    h     = relu(x @ W_eff^T + b0)        # [16384, 4096]
    out   = (h @ W2^T + b2).squeeze(-1)   # [16384]

Sharding: data-parallel over batch; each of the 8 cores handles 2048 rows of
x and replicates the weights. No collectives needed.

Per-core device kernel (bf16 operands, f32 PSUM):
  - W2 is folded into the weights on host (W0/B rows and b0 scaled by
    w2[m], m sign-sorted so positive-w2 rows come first).  Then
      out[b] = sum_P relu(z[b,m]) - sum_N relu(-z[b,m]) + b2,
    z = x @ (w2*W_eff)^T + w2*b0, i.e. fc2 collapses into a free-dim
    reduction that rides the RELU pass.
  - Layer 1 computes z tiles [128 batch, 1024 m] (two PSUM banks), PE
    accumulating 8 d-chunks per 512-wide half; lhsT = x^T slice
    [128d, 128b], rhs = W_eff^T block [128d, 512m].
  - LoRA rank-16 correction 2*(B@A)^T (w2-scaled) is added into the
    resident W_eff^T on device (PE matmul with zero-padded K operands ->
    PSUM, DVE add), just-in-time one m-pair phase ahead.
  - bias: one in-place DVE add per tile from a host-replicated w2*b0.
  - relu + fc2: ScalarE activation(Relu, scale=+-1, accum_out) per
    sign-pure m-segment; per-batch-row combine on DVE; final 128x16
    transpose on PE; single contiguous output DMA.
"""

import sys

sys.path.insert(0, "/opt/trn_rl_repo")

import ml_dtypes
import numpy as np

import concourse.bacc as bacc
import concourse.bass as bass
import concourse.mybir as mybir
import concourse.tile as tile
from concourse.bass_utils import run_bass_kernel_spmd

F32 = mybir.dt.float32
BF16 = mybir.dt.bfloat16
NP_BF16 = ml_dtypes.bfloat16

N_CORES = 8
B_FULL, D, M, R = 16384, 1024, 4096, 16
SCALING = 2.0
BS = B_FULL // N_CORES  # 2048 rows per core
NSUB = BS // 128  # 16 batch sub-chunks of 128
ND = D // 128  # 8 d-chunks
NMB = M // 512  # 8 m-blocks of 512
NMP = NMB // 2  # 4 m-pair phases of 1024

_CACHE = {}


def _act_plan(c):
    """Sign-pure activation segments per m-pair phase.

    Returns (plan, n_p, n_n): plan[mp] = list of (lo, hi, sign, is_p, col)
    with lo/hi in-pair column offsets, col the per-bsub accum column in
    RSP (is_p) or RSN.
    """
    plan = []
    p_cols = 0
    n_cols = 0
    for mp in range(NMP):
        start, end = mp * 1024, (mp + 1) * 1024
        segs = []
        if c >= end:
            segs.append((0, 1024, 1.0, True))
        elif c <= start:
            segs.append((0, 1024, -1.0, False))
        else:
            cb = c - start
            segs.append((0, cb, 1.0, True))
            segs.append((cb, 1024, -1.0, False))
        out = []
        for lo, hi, sign, is_p in segs:
            if is_p:
                out.append((lo, hi, sign, True, p_cols))
                p_cols += 1
            else:
                out.append((lo, hi, sign, False, n_cols))
                n_cols += 1
        plan.append(out)
    return plan, p_cols, n_cols


def _build_nc(c):
    plan, n_p, n_n = _act_plan(c)

    nc = bacc.Bacc(
        "TRN2",
        target_bir_lowering=False,
        debug=False,
        num_devices=N_CORES,
    )
    xt = nc.dram_tensor("xt", [128, NSUB * 1024], BF16, kind="ExternalInput").ap()
    w0t = nc.dram_tensor("w0t", [NMB, ND, 128, 512], BF16, kind="ExternalInput").ap()
    a2p = nc.dram_tensor("a2p", [128, D], BF16, kind="ExternalInput").ap()
    btp = nc.dram_tensor("btp", [128, M], BF16, kind="ExternalInput").ap()
    b0r = nc.dram_tensor("b0r", [128, M], BF16, kind="ExternalInput").ap()
    b2s = nc.dram_tensor("b2s", [128, 1], F32, kind="ExternalInput").ap()
    idn = nc.dram_tensor("idn", [128, 128], F32, kind="ExternalInput").ap()
    out = nc.dram_tensor("out", [NSUB, 128], F32, kind="ExternalOutput").ap()

    RELU = mybir.ActivationFunctionType.Relu
    ADD = mybir.AluOpType.add
    SUB = mybir.AluOpType.subtract
    MULT = mybir.AluOpType.mult
    AXX = mybir.AxisListType.X

    with tile.TileContext(nc) as tc:
        with (
            tc.tile_pool(name="cp", bufs=1) as cp,
            tc.tile_pool(name="hb", bufs=3) as hb,
            tc.tile_pool(name="rb", bufs=2) as rb,
            tc.tile_pool(name="psh", bufs=2, space="PSUM") as psh,
            tc.tile_pool(name="psl", bufs=2, space="PSUM") as psl,
            tc.tile_pool(name="pst", bufs=1, space="PSUM") as pst,
        ):
            A2 = cp.tile([128, D], BF16, tag="a2")
            BT = cp.tile([128, M], BF16, tag="bt")
            B0R = cp.tile([128, M], BF16, tag="b0r")
            W = cp.tile([128, NMB * ND * 512], BF16, tag="w")
            X = cp.tile([128, NSUB * 1024], BF16, tag="x")
            IDT = cp.tile([128, 128], F32, tag="idn")
            B2C = cp.tile([128, 1], F32, tag="b2")
            RSP = cp.tile([128, NSUB * max(n_p, 1)], F32, tag="rsp")
            RSN = cp.tile([128, NSUB * max(n_n, 1)], F32, tag="rsn")
            OUT = cp.tile([128, NSUB], F32, tag="out")

            def x_dma(b):
                nc.sync.dma_start(
                    out=X[:, b * 1024 : (b + 1) * 1024],
                    in_=xt[:, b * 1024 : (b + 1) * 1024],
                )

            def w_dma(mb, dc):
                off = (mb * ND + dc) * 512
                nc.sync.dma_start(out=W[:, off : off + 512], in_=w0t[mb, dc])

            def bt_dma(mb, eng):
                eng.dma_start(
                    out=BT[:, mb * 512 : (mb + 1) * 512],
                    in_=btp[:, mb * 512 : (mb + 1) * 512],
                )

            def lora_block(mb, dc):
                """W block (mb, dc) += w2-scaled 2*(B@A)^T, just-in-time."""
                off = (mb * ND + dc) * 512
                lp = psl.tile([128, 512], F32, tag="lp")
                nc.tensor.matmul(
                    lp[:],
                    A2[:, dc * 128 : (dc + 1) * 128],
                    BT[:, mb * 512 : (mb + 1) * 512],
                    start=True,
                    stop=True,
                )
                nc.vector.tensor_add(
                    W[:, off : off + 512], W[:, off : off + 512], lp[:]
                )

            # --- prologue DMAs: minimal working set first ---
            # A2 in two halves on the scalar queue (first lora blocks gate
            # on the first half only).
            nc.scalar.dma_start(out=A2[:, 0:512], in_=a2p[:, 0:512])
            bt_dma(0, nc.gpsimd)
            bt_dma(1, nc.gpsimd)
            nc.scalar.dma_start(out=A2[:, 512:1024], in_=a2p[:, 512:1024])
            x_dma(0)
            for dc in range(ND):
                w_dma(0, dc)
                w_dma(1, dc)
                if dc < 3:
                    x_dma(1 + dc)
            # LoRA for phase 0 (pair 0) right away.
            for dc in range(ND):
                lora_block(0, dc)
                lora_block(1, dc)
            nc.scalar.dma_start(
                out=B0R[:, 0:1024], in_=b0r[:, 0:1024]
            )

            # background loads issued on gpsimd queue (paced by position)
            nc.gpsimd.dma_start(out=B2C[:], in_=b2s)

            for mp in range(NMP):
                mb0, mb1 = 2 * mp, 2 * mp + 1
                nxt0, nxt1 = mb0 + 2, mb1 + 2
                for bsub in range(NSUB):
                    # paced background DMA / next-phase prep
                    if mp == 0:
                        if bsub < 12:
                            x_dma(4 + bsub)
                        elif bsub == 12:
                            nc.gpsimd.dma_start(out=IDT[:], in_=idn)
                    if mp < NMP - 1:
                        if bsub == 0:
                            bt_dma(nxt0, nc.gpsimd)
                            bt_dma(nxt1, nc.gpsimd)
                            nc.scalar.dma_start(
                                out=B0R[:, nxt0 * 512 : (nxt1 + 1) * 512],
                                in_=b0r[:, nxt0 * 512 : (nxt1 + 1) * 512],
                            )
                        if 1 <= bsub <= 8:
                            dc = bsub - 1
                            w_dma(nxt0, dc)
                            w_dma(nxt1, dc)
                        if 5 <= bsub <= 12:
                            dc = bsub - 5
                            lora_block(nxt0, dc)
                            lora_block(nxt1, dc)

                    hp = psh.tile([128, 1024], F32, tag="hp")
                    for dc in range(ND):
                        lhsT = X[:, bsub * 1024 + dc * 128 : bsub * 1024 + (dc + 1) * 128]
                        nc.tensor.matmul(
                            hp[:, 0:512],
                            lhsT,
                            W[:, (mb0 * ND + dc) * 512 : (mb0 * ND + dc) * 512 + 512],
                            start=(dc == 0),
                            stop=(dc == ND - 1),
                        )
                        nc.tensor.matmul(
                            hp[:, 512:1024],
                            lhsT,
                            W[:, (mb1 * ND + dc) * 512 : (mb1 * ND + dc) * 512 + 512],
                            start=(dc == 0),
                            stop=(dc == ND - 1),
                        )
                    # bias (w2-scaled b0) in place on the PSUM tile
                    nc.vector.tensor_add(
                        hp[:], hp[:], B0R[:, mp * 1024 : (mp + 1) * 1024]
                    )
                    # relu + fc2 partial sums, fused on ScalarE
                    for lo, hi, sign, is_p, col in plan[mp]:
                        hs = hb.tile([128, 1024], BF16, tag="hs")
                        rs = RSP if is_p else RSN
                        ncols = n_p if is_p else n_n
                        nc.scalar.activation(
                            hs[:, 0 : hi - lo],
                            hp[:, lo:hi],
                            RELU,
                            scale=sign,
                            accum_out=rs[:, bsub * ncols + col : bsub * ncols + col + 1],
                        )

            # --- combine: out[b] = sum_P - sum_N + b2, then transpose ---
            for bsub in range(NSUB):
                oc = OUT[:, bsub : bsub + 1]
                if n_p and n_n:
                    rp = rb.tile([128, 1], F32, tag="rp")
                    rn = rb.tile([128, 1], F32, tag="rn")
                    nc.vector.tensor_reduce(
                        rp[:], RSP[:, bsub * n_p : (bsub + 1) * n_p], AXX, ADD
                    )
                    nc.vector.tensor_reduce(
                        rn[:], RSN[:, bsub * n_n : (bsub + 1) * n_n], AXX, ADD
                    )
                    nc.vector.scalar_tensor_tensor(
                        oc, rp[:], B2C[:, 0:1], rn[:], ADD, SUB
                    )
                elif n_p:
                    rp = rb.tile([128, 1], F32, tag="rp")
                    nc.vector.tensor_reduce(
                        rp[:], RSP[:, bsub * n_p : (bsub + 1) * n_p], AXX, ADD
                    )
                    nc.vector.tensor_scalar_add(oc, rp[:], B2C[:, 0:1])
                else:
                    rn = rb.tile([128, 1], F32, tag="rn")
                    nc.vector.tensor_reduce(
                        rn[:], RSN[:, bsub * n_n : (bsub + 1) * n_n], AXX, ADD
                    )
                    nc.vector.scalar_tensor_tensor(
                        oc, rn[:], -1.0, B2C[:, 0:1], MULT, ADD
                    )
            pt = pst.tile([16, 128], F32, tag="pt")
            nc.tensor.transpose(pt[:], OUT[:, 0:NSUB], IDT[:])
            osb = rb.tile([16, 128], F32, tag="osb")
            nc.scalar.copy(osb[:], pt[:])
            nc.sync.dma_start(out=out, in_=osb[:])

    nc.compile()
    return nc


def _prep_in_maps(x, W0, b0, A, B, W2, b2, c, perm):
    w2 = W2[0]
    W0p = (W0 * w2[:, None])[perm]
    Bp = (B * w2[:, None])[perm]
    b0p = (b0 * w2)[perm]

    # [mb, dc, 128, 512]: block (mb, dc)[p, j] = W0p[mb*512 + j, dc*128 + p]
    w0t = np.ascontiguousarray(
        W0p.T.reshape(ND, 128, NMB, 512).transpose(2, 0, 1, 3)
    ).astype(NP_BF16)
    a2p = np.zeros((128, D), dtype=NP_BF16)
    a2p[:R] = (SCALING * A).astype(NP_BF16)
    btp = np.zeros((128, M), dtype=NP_BF16)
    btp[:R] = Bp.T.astype(NP_BF16)
    b0rep = np.ascontiguousarray(
        np.broadcast_to(b0p.astype(NP_BF16)[None, :], (128, M))
    )
    b2s = np.full((128, 1), b2[0], dtype=np.float32)
    idn = np.eye(128, dtype=np.float32)

    in_maps = []
    for cix in range(N_CORES):
        xs = x[cix * BS : (cix + 1) * BS]  # [2048, 1024]
        # xt[p, bsub*1024 + dc*128 + bb] = xs[bsub*128 + bb, dc*128 + p]
        xt = np.ascontiguousarray(
            xs.reshape(NSUB, 128, ND, 128).transpose(3, 0, 2, 1).reshape(128, NSUB * 1024)
        ).astype(NP_BF16)
        in_maps.append(
            {
                "xt": xt,
                "w0t": w0t,
                "a2p": a2p,
                "btp": btp,
                "b0r": b0rep,
                "b2s": b2s,
                "idn": idn,
            }
        )
    return in_maps


def kernel(x, W0, b0, A, B, W2, b2, _trace=False, _trace_kwargs=None):
    x = np.asarray(x, dtype=np.float32)
    W0 = np.asarray(W0, dtype=np.float32)
    b0 = np.asarray(b0, dtype=np.float32)
    A = np.asarray(A, dtype=np.float32)
    B = np.asarray(B, dtype=np.float32)
    W2 = np.asarray(W2, dtype=np.float32)
    b2 = np.asarray(b2, dtype=np.float32)

    w2 = W2[0]
    pos = w2 >= 0
    c = int(pos.sum())
    perm = np.concatenate([np.where(pos)[0], np.where(~pos)[0]])

    key = ("nc", c)
    if key not in _CACHE:
        _CACHE[key] = _build_nc(c)
    nc = _CACHE[key]

    in_maps = _prep_in_maps(x, W0, b0, A, B, W2, b2, c, perm)
    res = run_bass_kernel_spmd(
        nc,
        in_maps,
        list(range(N_CORES)),
        trace=_trace,
        **(_trace_kwargs or {}),
    )
    out = np.concatenate([r["out"].reshape(BS) for r in res.results])
    if _trace:
        _CACHE["last_results"] = res
    return out.astype(np.float32)
